# revision 18
# baseline (speedup 1.0000x reference)
"""Trainium2 Bass kernel for GQA multi-head attention (B=2, S=2048, D=2048,
16 Q heads / 4 KV heads, head_dim=128, RoPE, causal).

Sharding: 8 cores = 2 (batch) x 4 (tensor-parallel head groups).
Each core computes 4 Q heads + 1 KV head for one batch element and a partial
output projection; the host sums the 4 partials per batch element.

v3 design notes (all matmul operands bf16, fp32 PSUM/output):
  - weights are pre-laid-out on the host so each weight tensor loads with a
    few large DMAs ([128, n*chunk] with the d-chunk blocks along the free
    dim); x loads as 32 [128,1024] tiles resident across quarter pairs.
  - phase1: QT/KT = RoPE(wq/wk.T @ xT) in bf16 (negated-sin trick folds the
    rotate into one tensor_sub); V goes psum -> bf16 -> DMA-XBAR transpose
    into 16 [128,128] natural-layout tiles.
  - phase2 (per head, per 1024-wide q-half): scoresT = KT_c.T @ QT chunks in
    PSUM, exp on ACT -> at bf16, causal diag masked by DVE mul, AV accumulated
    over k-chunks on PE. Depth-2 software pipeline (AV lags scores by 2
    chunks) hides the PE->ACT->DVE->PE latency chain.
  - softmax denominator: at tiles are also accumulated across chunks on the
    DVE (bf16) and reduced with ONE [128,128] ones matmul per (head, half) --
    replaces the per-chunk ones matmuls (~29us of PE time).
  - phase3 (outT partial = wo_chunk.T @ AO) is emitted as filler inside the
    qh=1 attention stream so the PE chews on it whenever attention waits for
    the scalar engine's exp backlog; remaining pieces run as a pure-PE tail.
"""

import numpy as np
from contextlib import ExitStack

import ml_dtypes
import concourse.bass as bass
import concourse.mybir as mybir
from concourse import bacc, tile
from concourse.bass_utils import run_bass_kernel_spmd
from concourse.masks import make_upper_triangular

F32 = mybir.dt.float32
BF16 = mybir.dt.bfloat16
AF = mybir.ActivationFunctionType
NPBF16 = ml_dtypes.bfloat16

S = 2048
D = 2048
P = 128
NHQ = 4  # q heads per core
N_CORES = 8
N_TP = 4
PIPE_DEPTH = 3  # chunks of lookahead between scores and AV


def _slices512(rel, end):
    """Split [rel, end) into pieces that don't cross 512 (PSUM bank) bounds."""
    out = []
    o = rel
    while o < end:
        nxt = min(end, (o // 512 + 1) * 512)
        out.append((o, nxt))
        o = nxt
    return out


class _FillQueue:
    """Deferred emitters popped as PE filler inside the attention stream."""

    def __init__(self):
        self.items = []
        self._tick = 0

    def add(self, fn):
        self.items.append(fn)

    def pop_alternate(self):
        """Emit one item every other call (spreads items over the stream)."""
        self._tick += 1
        if self._tick % 2 == 0 and self.items:
            self.items.pop(0)()

    def drain(self):
        while self.items:
            self.items.pop(0)()


def _build_kernel(nc, tc, ctx, xT, wqc_d, wkvc_d, wof_d, cos2, sin2, outT):
    const = ctx.enter_context(tc.tile_pool(name="const", bufs=1))
    xtp = ctx.enter_context(tc.tile_pool(name="xtp", bufs=18))
    ropep = ctx.enter_context(tc.tile_pool(name="ropep", bufs=4))
    vtqp = ctx.enter_context(tc.tile_pool(name="vtqp", bufs=2))
    atp = ctx.enter_context(tc.tile_pool(name="atp", bufs=PIPE_DEPTH + 2))
    recp = ctx.enter_context(tc.tile_pool(name="recp", bufs=2))
    obp = ctx.enter_context(tc.tile_pool(name="obp", bufs=3))
    dena = ctx.enter_context(tc.tile_pool(name="dena", bufs=2))

    # ---- constants / persistent tensors ----
    wqc = const.tile([P, 16 * 512], BF16, tag="wqc")
    for i in range(4):
        nc.gpsimd.dma_start(wqc[:, 2048 * i:2048 * (i + 1)],
                            wqc_d[:, 2048 * i:2048 * (i + 1)])
    wkvc = const.tile([P, 16 * 256], BF16, tag="wkvc")
    for i in range(2):
        nc.gpsimd.dma_start(wkvc[:, 2048 * i:2048 * (i + 1)],
                            wkvc_d[:, 2048 * i:2048 * (i + 1)])
    wof = const.tile([P, 4 * 2048], BF16, tag="wof")
    for i in range(4):
        nc.gpsimd.dma_start(wof[:, 2048 * i:2048 * (i + 1)],
                            wof_d[:, 2048 * i:2048 * (i + 1)])
    cos2t = const.tile([P, S], F32, tag="cos2t")
    sin2t = const.tile([P, S], F32, tag="sin2t")
    nc.gpsimd.dma_start(cos2t[:], cos2[:])
    nc.gpsimd.dma_start(sin2t[:], sin2[:])

    maskt = const.tile([P, P], F32, tag="maskt")
    make_upper_triangular(nc, maskt[:], val=1.0, diag=True)
    onesb = const.tile([P, P], BF16, tag="onesb")
    nc.vector.memset(onesb[:], 1.0)

    QT = [const.tile([P, S], BF16, tag=f"QT{i}", name=f"QT{i}") for i in range(NHQ)]
    KT = const.tile([P, S], BF16, tag="KT")
    Vn = [const.tile([P, P], BF16, tag=f"Vn{j}", name=f"Vn{j}") for j in range(16)]
    AO = [const.tile([P, S], BF16, tag=f"AO{i}", name=f"AO{i}") for i in range(NHQ)]

    # ---- phase 1: projections + RoPE + V transpose ----
    with tc.tile_pool(name="ps1", bufs=1, space="PSUM") as ps1:
        _phase1(nc, ps1, xtp, ropep, vtqp, xT, wqc, wkvc, cos2t, sin2t, QT, KT, Vn)

    # ---- phase 2+3 interleaved ----
    with tc.tile_pool(name="ps23", bufs=1, space="PSUM") as ps23:
        # Touch the av/po tags before sc so sc's buffers land on the PSUM
        # banks phase1 frees earliest (or never used): the first score
        # matmuls then don't wait for the RoPE tail to drain.
        ps23.tile([P, 1024], F32, tag="av", bufs=1, name="avdummy")
        ps23.tile([P, 512], F32, tag="po", bufs=2, name="podummy")
        filler = _FillQueue()
        noop = _FillQueue()
        for h in range(NHQ):
            _attn_head(nc, ps23, atp, recp, dena, maskt, onesb, QT, KT, Vn,
                       AO, h, 0, noop)
        # qh=0 AO complete: queue phase3 half 0 as filler for the qh=1 stream
        for Dc in range(16):
            filler.add(_po_emitter(nc, ps23, obp, wof, AO, outT, Dc, 0, "dve"))
        for h in range(NHQ):
            _attn_head(nc, ps23, atp, recp, dena, maskt, onesb, QT, KT, Vn,
                       AO, h, 1, filler)
        filler.drain()
        for Dc in range(16):
            _po_emitter(nc, ps23, obp, wof, AO, outT, Dc, 1, "act")()


def _phase1(nc, ps1, xtp, ropep, vtqp, xT, wqc, wkvc, cos2t, sin2t, QT, KT, Vn):
    for sq2 in range(2):  # S-half: quarters 2*sq2, 2*sq2+1
        xts = []
        for dc in range(16):
            xt = xtp.tile([P, 1024], BF16)
            eng = nc.sync if dc % 2 == 0 else nc.scalar
            eng.dma_start(xt[:], xT[128 * dc:128 * (dc + 1),
                                    1024 * sq2:1024 * (sq2 + 1)])
            xts.append(xt)
        for half in range(2):
            sq = 2 * sq2 + half
            s0 = 512 * sq
            sl = slice(s0, s0 + 512)
            xsl = slice(512 * half, 512 * (half + 1))
            pQ = [ps1.tile([P, 512], F32, tag="acc", bufs=6, name=f"pQ{i}")
                  for i in range(NHQ)]
            pK = ps1.tile([P, 512], F32, tag="acc", bufs=6)
            pV = ps1.tile([P, 512], F32, tag="acc", bufs=6)
            for dc in range(16):
                st, sp = dc == 0, dc == 15
                for i in range(NHQ):
                    nc.tensor.matmul(
                        pQ[i][:],
                        wqc[:, 512 * dc + 128 * i:512 * dc + 128 * (i + 1)],
                        xts[dc][:, xsl], start=st, stop=sp,
                    )
                nc.tensor.matmul(pK[:], wkvc[:, 256 * dc:256 * dc + 128],
                                 xts[dc][:, xsl], start=st, stop=sp)
                nc.tensor.matmul(pV[:], wkvc[:, 256 * dc + 128:256 * dc + 256],
                                 xts[dc][:, xsl], start=st, stop=sp)
            # V: psum -> bf16 sbuf -> XBAR transpose to natural [s, dv] blocks
            # (emitted before the RoPE block: the copy is on ACT, so pV's
            # bank frees without waiting on the DVE queue)
            vq = vtqp.tile([P, 512], BF16)
            nc.scalar.activation(vq[:], pV[:], AF.Copy)
            for t in range(4):
                nc.scalar.dma_start_transpose(Vn[4 * sq + t][:],
                                              vq[:, 128 * t:128 * (t + 1)])
            # RoPE: rows 0:64 real, 64:128 imag (host deinterleaved); sin2t
            # rows 64:128 are pre-negated so one full-width sub finishes both
            # halves. PSUM operands of tensor_tensor may start at different
            # partitions (the half swap); SBUF operands share partition 0.
            # Last quarter: K first, so the banks the first score tile
            # aliases are released at the head of the DVE queue.
            ropelist = [(pQ[i], QT[i]) for i in range(NHQ)] + [(pK, KT)]
            if sq == 3:
                ropelist = ropelist[-1:] + ropelist[:-1]
            for psrc, dst in ropelist:
                m1 = ropep.tile([P, 512], F32, tag="m1")
                m2 = ropep.tile([P, 512], F32, tag="m2")
                nc.vector.tensor_mul(m1[:], psrc[:], cos2t[:, sl])
                nc.vector.tensor_mul(m2[0:64, :], psrc[64:128, :], sin2t[0:64, sl])
                nc.vector.tensor_mul(m2[64:128, :], psrc[0:64, :], sin2t[64:128, sl])
                nc.vector.tensor_sub(dst[:, sl], m1[:], m2[:])


def _attn_head(nc, ps, atp, recp, dena, maskt, onesb, QT, KT, Vn, AO,
               h, qh, filler):
    q0 = 1024 * qh
    cmax = (q0 + 1024 - 1) // 128
    pav = ps.tile([P, 1024], F32, tag="av", bufs=1)
    acc = dena.tile([P, 1024], BF16)
    pend = []
    for c in range(cmax + 1):
        k0 = 128 * c
        rel = max(q0, k0) - q0
        psc = ps.tile([P, 1024], F32, tag="sc", bufs=2)
        for o0, o1 in _slices512(rel, 1024):
            nc.tensor.matmul(
                psc[:, o0:o1], KT[:, k0:k0 + 128], QT[h][:, q0 + o0:q0 + o1],
                start=True, stop=True,
            )
        at = atp.tile([P, 1024], BF16)
        nc.scalar.activation(at[:, rel:1024], psc[:, rel:1024], AF.Exp)
        if k0 >= q0:  # diagonal block: causal 0/1 mask
            nc.vector.tensor_mul(at[:, rel:rel + 128], at[:, rel:rel + 128],
                                 maskt[:])
        # denominator accumulate (after mask)
        if c == 0:
            nc.vector.tensor_scalar_mul(acc[:], at[:], 1.0)
        else:
            nc.vector.tensor_add(acc[:, rel:1024], acc[:, rel:1024],
                                 at[:, rel:1024])
        pend.append((at, rel, c))
        if len(pend) > PIPE_DEPTH:
            _emit_av(nc, pav, Vn, *pend.pop(0), cmax)
            filler.pop_alternate()
    for p in pend:
        _emit_av(nc, pav, Vn, *p, cmax)
    # denominator: ones.T @ acc, one matmul pair per (head, half)
    pdn = ps.tile([P, 1024], F32, tag="sc", bufs=2)
    for o0 in (0, 512):
        nc.tensor.matmul(pdn[:, o0:o0 + 512], onesb[:], acc[:, o0:o0 + 512],
                         start=True, stop=True)
    rec = recp.tile([P, 1024], F32)
    nc.vector.reciprocal(rec[:], pdn[:])
    nc.vector.tensor_mul(AO[h][:, q0:q0 + 1024], pav[:], rec[:])


def _emit_av(nc, pav, Vn, at, rel, c, cmax):
    st, sp = c == 0, c == cmax
    for o0, o1 in _slices512(rel, 1024):
        nc.tensor.matmul(
            pav[:, o0:o1], Vn[c][:], at[:, o0:o1],
            start=st, stop=sp, skip_group_check=True,
        )


def _po_emitter(nc, ps, obp, wof, AO, outT, Dc, half, ob_eng):
    """Returns a closure emitting outT[Dc, half] = sum_h wo_h.T @ AO_h."""
    D0 = 128 * Dc

    def emit():
        ob = obp.tile([P, 1024], BF16)
        for j in range(2):
            o0 = 1024 * half + 512 * j
            po = ps.tile([P, 512], F32, tag="po", bufs=2)
            for hc in range(NHQ):
                nc.tensor.matmul(
                    po[:], wof[:, 2048 * hc + D0:2048 * hc + D0 + 128],
                    AO[hc][:, o0:o0 + 512],
                    start=(hc == 0), stop=(hc == 3), skip_group_check=True,
                )
            osl = slice(512 * j, 512 * (j + 1))
            if ob_eng == "dve":
                nc.vector.tensor_scalar_mul(ob[:, osl], po[:], 1.0)
            else:
                nc.scalar.activation(ob[:, osl], po[:], AF.Copy)
        nc.sync.dma_start(outT[D0:D0 + 128, 1024 * half:1024 * (half + 1)],
                          ob[:])

    return emit


_NC_CACHE = {}


def _get_nc(reps=1):
    """Build (and cache) the compiled Bass program. reps>1 wraps the whole
    body in a hardware loop -- used only by the timing harness to measure
    per-iteration execution time via wall-clock slope."""
    if reps in _NC_CACHE:
        return _NC_CACHE[reps]
    nc = bacc.Bacc("TRN2", target_bir_lowering=False, debug=False)
    aps = {}
    for name, shape, dt in [
        ("xT", [D, S], BF16), ("wqc", [P, 16 * 512], BF16),
        ("wkvc", [P, 16 * 256], BF16), ("wof", [P, 4 * 2048], BF16),
        ("cos2", [P, S], F32), ("sin2", [P, S], F32),
    ]:
        aps[name] = nc.dram_tensor(name, shape, dt, kind="ExternalInput").ap()
    outT = nc.dram_tensor("outT", [D, S], BF16, kind="ExternalOutput").ap()
    with tile.TileContext(nc) as tc, ExitStack() as ctx:
        if reps == 1:
            _build_kernel(
                nc, tc, ctx, aps["xT"], aps["wqc"], aps["wkvc"], aps["wof"],
                aps["cos2"], aps["sin2"], outT,
            )
        else:
            with tc.For_i(0, reps, 1):
                with ExitStack() as inner:
                    _build_kernel(
                        nc, tc, inner, aps["xT"], aps["wqc"], aps["wkvc"],
                        aps["wof"], aps["cos2"], aps["sin2"], outT,
                    )
    nc.compile()
    _NC_CACHE[reps] = nc
    return nc


def _chunked_rows(a, n_chunk):
    """[n_chunk*128, W] -> [128, n_chunk*W] with chunk blocks along free."""
    W = a.shape[1]
    return np.ascontiguousarray(
        a.reshape(n_chunk, P, W).transpose(1, 0, 2).reshape(P, n_chunk * W)
    )


def _prep_in_maps(x, freqs_cos, freqs_sin, w_q, w_k, w_v, w_o):
    x = np.asarray(x, np.float32)
    cosT = np.asarray(freqs_cos, np.float32).T  # [64, S]
    sinT = np.asarray(freqs_sin, np.float32).T
    cos2 = np.ascontiguousarray(np.concatenate([cosT, cosT], 0))
    sin2 = np.ascontiguousarray(np.concatenate([sinT, -sinT], 0))
    w_q = np.asarray(w_q, np.float32)
    w_k = np.asarray(w_k, np.float32)
    w_v = np.asarray(w_v, np.float32)
    w_o = np.asarray(w_o, np.float32)

    # deinterleave head_dim: evens then odds (consistent for q and k)
    perm1 = np.concatenate([np.arange(0, P, 2), np.arange(1, P, 2)])
    in_maps = []
    for core in range(N_CORES):
        b, tp = divmod(core, N_TP)
        qcols = np.concatenate(
            [4 * tp * P + h * P + perm1 for h in range(NHQ)]
        )
        kcols = tp * P + perm1
        wqc = _chunked_rows(w_q[:, qcols] * (P ** -0.5), 16).astype(NPBF16)
        wkvc = _chunked_rows(np.concatenate(
            [w_k[:, kcols], w_v[:, tp * P:(tp + 1) * P]], axis=1), 16
        ).astype(NPBF16)
        wof = _chunked_rows(w_o[4 * tp * P:4 * (tp + 1) * P, :], 4).astype(NPBF16)
        xT = np.ascontiguousarray(x[b].T).astype(NPBF16)
        in_maps.append({
            "xT": xT, "wqc": wqc, "wkvc": wkvc, "wof": wof,
            "cos2": cos2, "sin2": sin2,
        })
    return in_maps


def kernel(x, freqs_cos, freqs_sin, w_q, w_k, w_v, w_o):
    nc = _get_nc()
    in_maps = _prep_in_maps(x, freqs_cos, freqs_sin, w_q, w_k, w_v, w_o)
    results = run_bass_kernel_spmd(nc, in_maps, list(range(N_CORES))).results
    B = 2
    out = np.zeros((B, S, D), np.float32)
    for core in range(N_CORES):
        out[core // N_TP] += results[core]["outT"].T
    return out


# revision 20
# speedup vs baseline: 1.0033x; 1.0033x over previous
"""Trainium2 Bass kernel for GQA multi-head attention (B=2, S=2048, D=2048,
16 Q heads / 4 KV heads, head_dim=128, RoPE, causal).

Sharding: 8 cores = 2 (batch) x 4 (tensor-parallel head groups).
Each core computes 4 Q heads + 1 KV head for one batch element and a partial
output projection; the host sums the 4 partials per batch element.

v3 design notes (all matmul operands bf16, fp32 PSUM/output):
  - weights are pre-laid-out on the host so each weight tensor loads with a
    few large DMAs ([128, n*chunk] with the d-chunk blocks along the free
    dim); x loads as 32 [128,1024] tiles resident across quarter pairs.
  - phase1: QT/KT = RoPE(wq/wk.T @ xT) in bf16 (negated-sin trick folds the
    rotate into one tensor_sub); V goes psum -> bf16 -> DMA-XBAR transpose
    into 16 [128,128] natural-layout tiles.
  - phase2 (per head, per 1024-wide q-half): scoresT = KT_c.T @ QT chunks in
    PSUM, exp on ACT -> at bf16, causal diag masked by DVE mul, AV accumulated
    over k-chunks on PE. Depth-2 software pipeline (AV lags scores by 2
    chunks) hides the PE->ACT->DVE->PE latency chain.
  - softmax denominator: at tiles are also accumulated across chunks on the
    DVE (bf16) and reduced with ONE [128,128] ones matmul per (head, half) --
    replaces the per-chunk ones matmuls (~29us of PE time).
  - phase3 (outT partial = wo_chunk.T @ AO) is emitted as filler inside the
    qh=1 attention stream so the PE chews on it whenever attention waits for
    the scalar engine's exp backlog; remaining pieces run as a pure-PE tail.
"""

import numpy as np
from contextlib import ExitStack

import ml_dtypes
import concourse.bass as bass
import concourse.mybir as mybir
from concourse import bacc, tile
from concourse.bass_utils import run_bass_kernel_spmd
from concourse.masks import make_upper_triangular

F32 = mybir.dt.float32
BF16 = mybir.dt.bfloat16
AF = mybir.ActivationFunctionType
NPBF16 = ml_dtypes.bfloat16

S = 2048
D = 2048
P = 128
NHQ = 4  # q heads per core
N_CORES = 8
N_TP = 4
PIPE_DEPTH = 3  # chunks of lookahead between scores and AV


def _slices512(rel, end):
    """Split [rel, end) into pieces that don't cross 512 (PSUM bank) bounds."""
    out = []
    o = rel
    while o < end:
        nxt = min(end, (o // 512 + 1) * 512)
        out.append((o, nxt))
        o = nxt
    return out


class _FillQueue:
    """Deferred emitters popped as PE filler inside the attention stream."""

    def __init__(self):
        self.items = []
        self._tick = 0

    def add(self, fn):
        self.items.append(fn)

    def pop_alternate(self):
        """Emit one item every other call (spreads items over the stream)."""
        self._tick += 1
        if self._tick % 2 == 0 and self.items:
            self.items.pop(0)()

    def drain(self):
        while self.items:
            self.items.pop(0)()


def _build_kernel(nc, tc, ctx, xT, wqc_d, wkvc_d, wof_d, cos2, sin2, outT):
    const = ctx.enter_context(tc.tile_pool(name="const", bufs=1))
    xtp = ctx.enter_context(tc.tile_pool(name="xtp", bufs=18))
    ropep = ctx.enter_context(tc.tile_pool(name="ropep", bufs=4))
    vtqp = ctx.enter_context(tc.tile_pool(name="vtqp", bufs=2))
    atp = ctx.enter_context(tc.tile_pool(name="atp", bufs=PIPE_DEPTH + 2))
    recp = ctx.enter_context(tc.tile_pool(name="recp", bufs=2))
    obp = ctx.enter_context(tc.tile_pool(name="obp", bufs=3))
    dena = ctx.enter_context(tc.tile_pool(name="dena", bufs=2))

    # ---- constants / persistent tensors ----
    wqc = const.tile([P, 16 * 512], BF16, tag="wqc")
    for i in range(4):
        nc.gpsimd.dma_start(wqc[:, 2048 * i:2048 * (i + 1)],
                            wqc_d[:, 2048 * i:2048 * (i + 1)])
    wkvc = const.tile([P, 16 * 256], BF16, tag="wkvc")
    for i in range(2):
        nc.gpsimd.dma_start(wkvc[:, 2048 * i:2048 * (i + 1)],
                            wkvc_d[:, 2048 * i:2048 * (i + 1)])
    wof = const.tile([P, 4 * 2048], BF16, tag="wof")
    for i in range(4):
        nc.gpsimd.dma_start(wof[:, 2048 * i:2048 * (i + 1)],
                            wof_d[:, 2048 * i:2048 * (i + 1)])
    cos2t = const.tile([P, S], F32, tag="cos2t")
    sin2t = const.tile([P, S], F32, tag="sin2t")
    nc.gpsimd.dma_start(cos2t[:], cos2[:])
    nc.gpsimd.dma_start(sin2t[:], sin2[:])

    maskt = const.tile([P, P], F32, tag="maskt")
    make_upper_triangular(nc, maskt[:], val=1.0, diag=True)
    onesb = const.tile([P, P], BF16, tag="onesb")
    nc.vector.memset(onesb[:], 1.0)

    QT = [const.tile([P, S], BF16, tag=f"QT{i}", name=f"QT{i}") for i in range(NHQ)]
    KT = const.tile([P, S], BF16, tag="KT")
    Vn = [const.tile([P, P], BF16, tag=f"Vn{j}", name=f"Vn{j}") for j in range(16)]
    AO = [const.tile([P, S], BF16, tag=f"AO{i}", name=f"AO{i}") for i in range(NHQ)]

    # ---- phase 1: projections + RoPE + V transpose ----
    with tc.tile_pool(name="ps1", bufs=1, space="PSUM") as ps1:
        _phase1(nc, ps1, xtp, ropep, vtqp, xT, wqc, wkvc, cos2t, sin2t, QT, KT, Vn)

    # ---- phase 2+3 interleaved ----
    with tc.tile_pool(name="ps23", bufs=1, space="PSUM") as ps23:
        filler = _FillQueue()
        noop = _FillQueue()
        for h in range(NHQ):
            _attn_head(nc, ps23, atp, recp, dena, maskt, onesb, QT, KT, Vn,
                       AO, h, 0, noop)
        # qh=0 AO complete: queue phase3 half 0 as filler for the qh=1 stream
        for Dc in range(16):
            filler.add(_po_emitter(nc, ps23, obp, wof, AO, outT, Dc, 0, "dve"))
        for h in range(NHQ):
            _attn_head(nc, ps23, atp, recp, dena, maskt, onesb, QT, KT, Vn,
                       AO, h, 1, filler)
        filler.drain()
        for Dc in range(16):
            _po_emitter(nc, ps23, obp, wof, AO, outT, Dc, 1, "act")()


def _phase1(nc, ps1, xtp, ropep, vtqp, xT, wqc, wkvc, cos2t, sin2t, QT, KT, Vn):
    for sq2 in range(2):  # S-half: quarters 2*sq2, 2*sq2+1
        xts = []
        for dc in range(16):
            xt = xtp.tile([P, 1024], BF16)
            eng = nc.sync if dc % 2 == 0 else nc.scalar
            eng.dma_start(xt[:], xT[128 * dc:128 * (dc + 1),
                                    1024 * sq2:1024 * (sq2 + 1)])
            xts.append(xt)
        for half in range(2):
            sq = 2 * sq2 + half
            s0 = 512 * sq
            sl = slice(s0, s0 + 512)
            xsl = slice(512 * half, 512 * (half + 1))
            pQ = [ps1.tile([P, 512], F32, tag="acc", bufs=6, name=f"pQ{i}")
                  for i in range(NHQ)]
            pK = ps1.tile([P, 512], F32, tag="acc", bufs=6)
            pV = ps1.tile([P, 512], F32, tag="acc", bufs=6)
            for dc in range(16):
                st, sp = dc == 0, dc == 15
                for i in range(NHQ):
                    nc.tensor.matmul(
                        pQ[i][:],
                        wqc[:, 512 * dc + 128 * i:512 * dc + 128 * (i + 1)],
                        xts[dc][:, xsl], start=st, stop=sp,
                    )
                nc.tensor.matmul(pK[:], wkvc[:, 256 * dc:256 * dc + 128],
                                 xts[dc][:, xsl], start=st, stop=sp)
                nc.tensor.matmul(pV[:], wkvc[:, 256 * dc + 128:256 * dc + 256],
                                 xts[dc][:, xsl], start=st, stop=sp)
            # V: psum -> bf16 sbuf -> XBAR transpose to natural [s, dv] blocks
            # (emitted before the RoPE block: the copy is on ACT, so pV's
            # bank frees without waiting on the DVE queue)
            vq = vtqp.tile([P, 512], BF16)
            nc.scalar.activation(vq[:], pV[:], AF.Copy)
            for t in range(4):
                nc.scalar.dma_start_transpose(Vn[4 * sq + t][:],
                                              vq[:, 128 * t:128 * (t + 1)])
            # RoPE: rows 0:64 real, 64:128 imag (host deinterleaved); sin2t
            # rows 64:128 are pre-negated so one full-width sub finishes both
            # halves. PSUM operands of tensor_tensor may start at different
            # partitions (the half swap); SBUF operands share partition 0.
            for psrc, dst in [(pQ[i], QT[i]) for i in range(NHQ)] + [(pK, KT)]:
                m1 = ropep.tile([P, 512], F32, tag="m1")
                m2 = ropep.tile([P, 512], F32, tag="m2")
                nc.vector.tensor_mul(m1[:], psrc[:], cos2t[:, sl])
                nc.vector.tensor_mul(m2[0:64, :], psrc[64:128, :], sin2t[0:64, sl])
                nc.vector.tensor_mul(m2[64:128, :], psrc[0:64, :], sin2t[64:128, sl])
                nc.vector.tensor_sub(dst[:, sl], m1[:], m2[:])


def _attn_head(nc, ps, atp, recp, dena, maskt, onesb, QT, KT, Vn, AO,
               h, qh, filler):
    q0 = 1024 * qh
    cmax = (q0 + 1024 - 1) // 128
    pav = ps.tile([P, 1024], F32, tag="av", bufs=1)
    acc = dena.tile([P, 1024], BF16)
    pend = []
    for c in range(cmax + 1):
        k0 = 128 * c
        rel = max(q0, k0) - q0
        psc = ps.tile([P, 1024], F32, tag="sc", bufs=2)
        for o0, o1 in _slices512(rel, 1024):
            nc.tensor.matmul(
                psc[:, o0:o1], KT[:, k0:k0 + 128], QT[h][:, q0 + o0:q0 + o1],
                start=True, stop=True,
            )
        at = atp.tile([P, 1024], BF16)
        nc.scalar.activation(at[:, rel:1024], psc[:, rel:1024], AF.Exp)
        if k0 >= q0:  # diagonal block: causal 0/1 mask
            nc.vector.tensor_mul(at[:, rel:rel + 128], at[:, rel:rel + 128],
                                 maskt[:])
        # denominator accumulate (after mask)
        if c == 0:
            nc.vector.tensor_scalar_mul(acc[:], at[:], 1.0)
        else:
            nc.vector.tensor_add(acc[:, rel:1024], acc[:, rel:1024],
                                 at[:, rel:1024])
        pend.append((at, rel, c))
        if len(pend) > PIPE_DEPTH:
            _emit_av(nc, pav, Vn, *pend.pop(0), cmax)
            filler.pop_alternate()
    for p in pend:
        _emit_av(nc, pav, Vn, *p, cmax)
    # denominator: ones.T @ acc, one matmul pair per (head, half)
    pdn = ps.tile([P, 1024], F32, tag="sc", bufs=2)
    for o0 in (0, 512):
        nc.tensor.matmul(pdn[:, o0:o0 + 512], onesb[:], acc[:, o0:o0 + 512],
                         start=True, stop=True)
    rec = recp.tile([P, 1024], F32)
    nc.vector.reciprocal(rec[:], pdn[:])
    nc.vector.tensor_mul(AO[h][:, q0:q0 + 1024], pav[:], rec[:])


def _emit_av(nc, pav, Vn, at, rel, c, cmax):
    st, sp = c == 0, c == cmax
    for o0, o1 in _slices512(rel, 1024):
        nc.tensor.matmul(
            pav[:, o0:o1], Vn[c][:], at[:, o0:o1],
            start=st, stop=sp, skip_group_check=True,
        )


def _po_emitter(nc, ps, obp, wof, AO, outT, Dc, half, ob_eng):
    """Returns a closure emitting outT[Dc, half] = sum_h wo_h.T @ AO_h."""
    D0 = 128 * Dc

    def emit():
        ob = obp.tile([P, 1024], BF16)
        for j in range(2):
            o0 = 1024 * half + 512 * j
            po = ps.tile([P, 512], F32, tag="po", bufs=2)
            for hc in range(NHQ):
                nc.tensor.matmul(
                    po[:], wof[:, 2048 * hc + D0:2048 * hc + D0 + 128],
                    AO[hc][:, o0:o0 + 512],
                    start=(hc == 0), stop=(hc == 3), skip_group_check=True,
                )
            osl = slice(512 * j, 512 * (j + 1))
            if ob_eng == "dve":
                nc.vector.tensor_scalar_mul(ob[:, osl], po[:], 1.0)
            else:
                nc.scalar.activation(ob[:, osl], po[:], AF.Copy)
        nc.sync.dma_start(outT[D0:D0 + 128, 1024 * half:1024 * (half + 1)],
                          ob[:])

    return emit


_NC_CACHE = {}


def _get_nc(reps=1):
    """Build (and cache) the compiled Bass program. reps>1 wraps the whole
    body in a hardware loop -- used only by the timing harness to measure
    per-iteration execution time via wall-clock slope."""
    if reps in _NC_CACHE:
        return _NC_CACHE[reps]
    nc = bacc.Bacc("TRN2", target_bir_lowering=False, debug=False)
    aps = {}
    for name, shape, dt in [
        ("xT", [D, S], BF16), ("wqc", [P, 16 * 512], BF16),
        ("wkvc", [P, 16 * 256], BF16), ("wof", [P, 4 * 2048], BF16),
        ("cos2", [P, S], F32), ("sin2", [P, S], F32),
    ]:
        aps[name] = nc.dram_tensor(name, shape, dt, kind="ExternalInput").ap()
    outT = nc.dram_tensor("outT", [D, S], BF16, kind="ExternalOutput").ap()
    with tile.TileContext(nc) as tc, ExitStack() as ctx:
        if reps == 1:
            _build_kernel(
                nc, tc, ctx, aps["xT"], aps["wqc"], aps["wkvc"], aps["wof"],
                aps["cos2"], aps["sin2"], outT,
            )
        else:
            with tc.For_i(0, reps, 1):
                with ExitStack() as inner:
                    _build_kernel(
                        nc, tc, inner, aps["xT"], aps["wqc"], aps["wkvc"],
                        aps["wof"], aps["cos2"], aps["sin2"], outT,
                    )
    nc.compile()
    _NC_CACHE[reps] = nc
    return nc


def _chunked_rows(a, n_chunk):
    """[n_chunk*128, W] -> [128, n_chunk*W] with chunk blocks along free."""
    W = a.shape[1]
    return np.ascontiguousarray(
        a.reshape(n_chunk, P, W).transpose(1, 0, 2).reshape(P, n_chunk * W)
    )


def _prep_in_maps(x, freqs_cos, freqs_sin, w_q, w_k, w_v, w_o):
    x = np.asarray(x, np.float32)
    cosT = np.asarray(freqs_cos, np.float32).T  # [64, S]
    sinT = np.asarray(freqs_sin, np.float32).T
    cos2 = np.ascontiguousarray(np.concatenate([cosT, cosT], 0))
    sin2 = np.ascontiguousarray(np.concatenate([sinT, -sinT], 0))
    w_q = np.asarray(w_q, np.float32)
    w_k = np.asarray(w_k, np.float32)
    w_v = np.asarray(w_v, np.float32)
    w_o = np.asarray(w_o, np.float32)

    # deinterleave head_dim: evens then odds (consistent for q and k)
    perm1 = np.concatenate([np.arange(0, P, 2), np.arange(1, P, 2)])
    in_maps = []
    for core in range(N_CORES):
        b, tp = divmod(core, N_TP)
        qcols = np.concatenate(
            [4 * tp * P + h * P + perm1 for h in range(NHQ)]
        )
        kcols = tp * P + perm1
        wqc = _chunked_rows(w_q[:, qcols] * (P ** -0.5), 16).astype(NPBF16)
        wkvc = _chunked_rows(np.concatenate(
            [w_k[:, kcols], w_v[:, tp * P:(tp + 1) * P]], axis=1), 16
        ).astype(NPBF16)
        wof = _chunked_rows(w_o[4 * tp * P:4 * (tp + 1) * P, :], 4).astype(NPBF16)
        xT = np.ascontiguousarray(x[b].T).astype(NPBF16)
        in_maps.append({
            "xT": xT, "wqc": wqc, "wkvc": wkvc, "wof": wof,
            "cos2": cos2, "sin2": sin2,
        })
    return in_maps


def kernel(x, freqs_cos, freqs_sin, w_q, w_k, w_v, w_o):
    nc = _get_nc()
    in_maps = _prep_in_maps(x, freqs_cos, freqs_sin, w_q, w_k, w_v, w_o)
    results = run_bass_kernel_spmd(nc, in_maps, list(range(N_CORES))).results
    B = 2
    out = np.zeros((B, S, D), np.float32)
    for core in range(N_CORES):
        out[core // N_TP] += results[core]["outT"].T
    return out
